# revision 30
# baseline (speedup 1.0000x reference)
"""Trainium2 Bass kernel for EvolveGCN-O forward (GCN message passing).

Math (reference):
    h   = x @ Wp + bp
    W   = LSTM-evolved weight from initial_weight (one step, h0=c0=IW)
    hw  = h @ W
    out = D^-1/2 (A+I) D^-1/2 hw + b_gcn

Factored for the kernel:
    out[d] = dinv[d] * (sum_{e: dst=d} dinv[src_e] * x[src_e]) @ (Wp @ W)
             + s2[d]*dinv[d]*(bp @ W) + b_gcn
with s2[d] = sum_{e in(d)} dinv[src_e] (self loops included as edges).

Distribution: nodes (dsts) sharded over 8 NeuronCores (serpentine by degree).
The aggregation over in-edges is computed as a dense blocked matmul: for each
source rank r (128 nodes), xaggT[:, :] += XsT_r @ M_r where Xs is the
dinv-scaled fp16 source-feature table (replicated) and M_r is the fp8 block of
the edge-multiplicity matrix (src-rank r x this core's 1280 dsts; counts are
small integers, exact in fp8). M is streamed from HBM in rank chunks while the
TensorEngine accumulates all 80 ranks into PSUM; no per-edge DMA is needed.
The tiny [H,H] LSTM weight evolution is replicated on every core.
"""

import numpy as np

N_NODES = 10000
N_EDGES = 320000
IN_DIM = 128
HID = 256
M = 8                    # NeuronCores
NP = 10240               # padded node count (mult of 128)
RANKS = NP // 128        # 80
NPC = NP // M            # 1280 padded dsts per core
NGRP = NPC // 128        # 10 dst blocks of 128 per core
RCH = 8                  # ranks per streamed M chunk
NCHK = RANKS // RCH      # 10 chunks

_cache = {}


def _build_module():
    """Build+compile the Bacc module (shapes are static)."""
    import concourse.bacc as bacc
    import concourse.mybir as mybir
    import concourse.tile as tile

    nc = bacc.Bacc("TRN2", target_bir_lowering=False, debug=False,
                   num_devices=M)
    f32, f16, f8 = mybir.dt.float32, mybir.dt.float16, mybir.dt.float8e4
    bf16 = mybir.dt.bfloat16

    # ---- DRAM inputs ----
    xs_in = nc.dram_tensor("xs_tiled", [128, RANKS * 128], f16, kind="ExternalInput").ap()
    m_in = nc.dram_tensor("Mt", [128, RANKS * NPC], f8, kind="ExternalInput").ap()
    ws_in = nc.dram_tensor("WsumT", [256, 1024], bf16, kind="ExternalInput").ap()
    iw_in = nc.dram_tensor("IW", [256, 256], f32, kind="ExternalInput").ap()
    iwt_in = nc.dram_tensor("IWT", [256, 256], bf16, kind="ExternalInput").ap()
    wpt_in = nc.dram_tensor("WpT", [256, 128], f32, kind="ExternalInput").ap()
    bsum_in = nc.dram_tensor("bsum", [1, 1024], f32, kind="ExternalInput").ap()
    bp_in = nc.dram_tensor("bp_col", [256, 1], f32, kind="ExternalInput").ap()
    bgcn_in = nc.dram_tensor("b_gcn", [1, 256], f32, kind="ExternalInput").ap()
    ones_in = nc.dram_tensor("ones_row", [1, 128], f32, kind="ExternalInput").ap()
    s2d_in = nc.dram_tensor("s2d_col", [128, NGRP], f32, kind="ExternalInput").ap()
    dcol_in = nc.dram_tensor("dinv_col", [128, NGRP], f32, kind="ExternalInput").ap()

    out_t = nc.dram_tensor("out", [NPC, HID], f32, kind="ExternalOutput").ap()

    with tile.TileContext(nc) as tc:
        with (
            tc.tile_pool(name="stage", bufs=1) as stpool,
            tc.tile_pool(name="persist", bufs=1) as pp,
            tc.tile_pool(name="mp", bufs=4) as mpool,
            tc.tile_pool(name="op", bufs=3) as opool,
            tc.tile_pool(name="psa", bufs=1, space="PSUM") as psa,
            tc.tile_pool(name="psg", bufs=2, space="PSUM") as psg,
            tc.tile_pool(name="psl", bufs=1, space="PSUM") as psl,
        ):
            # ---------- tiny loads first; PE warmup to raise the clock pstate --
            bsum = pp.tile([1, 1024], f32)
            bgcn = pp.tile([1, 256], f32)
            ones = pp.tile([1, 128], f32)
            # PE warmup on framework consts (no data deps) to kick the clock
            # pstate ramp as early as possible
            cw = nc.const_aps.tensor(1.0, [128, 8])
            cw1 = nc.const_aps.tensor(1.0, [128, 1])
            wu_ps = psg.tile([8, 1], f32, space="PSUM", tag="ops")
            for _ in range(40):
                nc.tensor.matmul(out=wu_ps[:], lhsT=cw, rhs=cw1,
                                 start=True, stop=True)

            # tiny+weight loads all on the scalar queue; sync starts xs0+M0
            nc.scalar.dma_start(out=ones[:], in_=ones_in[:])
            nc.scalar.dma_start(out=bsum[:], in_=bsum_in[:])
            nc.scalar.dma_start(out=bgcn[:], in_=bgcn_in[:])

            wsum = pp.tile([128, 2, 1024], bf16)
            iwt = pp.tile([128, 2, 256], bf16)
            iw = pp.tile([128, 2, 256], f32)
            wpt = pp.tile([128, 2, 128], f32)
            bp_c = pp.tile([128, 2, 1], f32)
            nc.scalar.dma_start(out=wsum[:], in_=ws_in.rearrange("(k p) c -> p k c", p=128))
            nc.scalar.dma_start(out=iwt[:], in_=iwt_in.rearrange("(k p) c -> p k c", p=128))

            # ---------- stream xs + M per chunk; PE chases ----------
            xs_sb = pp.tile([128, RANKS * 128], f16)
            xagg = pp.tile([128, NPC], bf16)
            pa0 = psa.tile([128, 512], f32, space="PSUM", tag="pa0")
            pa1 = psa.tile([128, 512], f32, space="PSUM", tag="pa1")
            pa2 = psa.tile([128, 256], f32, space="PSUM", tag="pa2")
            pa = [pa0, pa1, pa2]
            spans = [(0, 512), (512, 1024), (1024, 1280)]
            w_ev = pp.tile([128, 2, 256], f32)   # evolved GCN weight W
            wpw = pp.tile([128, 256], bf16)      # Wp @ W
            bpwf = pp.tile([128, 256], f32)      # bp @ W, replicated rows
            t2 = pp.tile([128, NGRP, 256], f32)  # s2*dinv*bpW + b_gcn per block
            s2d = pp.tile([128, NGRP], f32)
            dcol = pp.tile([128, NGRP], f32)
            Sig = mybir.ActivationFunctionType.Sigmoid
            Tanh = mybir.ActivationFunctionType.Tanh

            def emit_gates(ic):
                # LSTM gates for IW row-chunk ic -> w_ev[:, ic, :]
                for h in range(2):
                    gpsum = psl.tile([128, 512], f32, space="PSUM", tag="gates")
                    gs = slice(512 * h, 512 * (h + 1))
                    nc.tensor.matmul(out=gpsum[:], lhsT=ones[:, :],
                                     rhs=bsum[:, gs], start=True, stop=False)
                    nc.tensor.matmul(out=gpsum[:],
                                     lhsT=iwt[:, 0, 128 * ic:128 * (ic + 1)],
                                     rhs=wsum[:, 0, gs], start=False, stop=False)
                    nc.tensor.matmul(out=gpsum[:],
                                     lhsT=iwt[:, 1, 128 * ic:128 * (ic + 1)],
                                     rhs=wsum[:, 1, gs], start=False, stop=True)
                    a0 = stpool.tile([128, 256], f32, tag=f"a{2*h}")
                    a1 = stpool.tile([128, 256], f32, tag=f"a{2*h+1}")
                    nc.scalar.activation(out=a0[:], in_=gpsum[:, 0:256],
                                         func=(Sig if h == 0 else Tanh))
                    nc.scalar.activation(out=a1[:], in_=gpsum[:, 256:512], func=Sig)
                    if h == 0:
                        si, sf = a0, a1
                    else:
                        tg, so = a0, a1
                c1 = stpool.tile([128, 256], f32, tag="c1")
                nc.vector.tensor_tensor(out=c1[:], in0=sf[:], in1=iw[:, ic, :],
                                        op=mybir.AluOpType.mult)
                c2 = stpool.tile([128, 256], f32, tag="c2")
                nc.vector.tensor_tensor(out=c2[:], in0=si[:], in1=tg[:],
                                        op=mybir.AluOpType.mult)
                cc = stpool.tile([128, 256], f32, tag="cc")
                nc.vector.tensor_tensor(out=cc[:], in0=c1[:], in1=c2[:],
                                        op=mybir.AluOpType.add)
                tcc = stpool.tile([128, 256], f32, tag="tcc")
                nc.scalar.activation(out=tcc[:], in_=cc[:], func=Tanh)
                nc.vector.tensor_tensor(out=w_ev[:, ic, :], in0=so[:], in1=tcc[:],
                                        op=mybir.AluOpType.mult)

            nc.scalar.dma_start(
                out=iw[:], in_=iw_in.rearrange("(k p) c -> p k c", p=128))
            nc.scalar.dma_start(
                out=wpt[:], in_=wpt_in.rearrange("(k p) c -> p k c", p=128))
            nc.scalar.dma_start(
                out=bp_c[:], in_=bp_in.rearrange("(k p) c -> p k c", p=128))
            nc.scalar.dma_start(out=s2d[:], in_=s2d_in[:])
            nc.scalar.dma_start(out=dcol[:], in_=dcol_in[:])
            emit_gates(0)
            emit_gates(1)

            for c in range(NCHK):
                eng = nc.sync if c % 2 == 0 else nc.scalar
                xsl = slice(c * RCH * 128, (c + 1) * RCH * 128)
                eng.dma_start(out=xs_sb[:, xsl], in_=xs_in[:, xsl])
                mt = mpool.tile([128, RCH, NPC], f8, tag="mt")
                eng.dma_start(
                    out=mt[:],
                    in_=m_in[:, c * RCH * NPC:(c + 1) * RCH * NPC]
                        .rearrange("p (r d) -> p r d", d=NPC))
                for k in range(RCH):
                    r = c * RCH + k
                    lhsT = xs_sb[:, r * 128:(r + 1) * 128]
                    for t in range(3):
                        lo, hi = spans[t]
                        nc.tensor.matmul(out=pa[t][:], lhsT=lhsT,
                                         rhs=mt[:, k, lo:hi],
                                         start=(r == 0), stop=(r == RANKS - 1))
                # small-tensor work rides under the stream
                if c == 0:
                    wp_ps = psl.tile([128, 256], f32, space="PSUM", tag="gates")
                    nc.tensor.matmul(out=wp_ps[:], lhsT=wpt[:, 0, :],
                                     rhs=w_ev[:, 0, :], start=True, stop=False)
                    nc.tensor.matmul(out=wp_ps[:], lhsT=wpt[:, 1, :],
                                     rhs=w_ev[:, 1, :], start=False, stop=True)
                    nc.vector.tensor_copy(out=wpw[:], in_=wp_ps[:])
                elif c == 1:
                    # bp @ W replicated to all 128 partitions:
                    # bpwf = ones128 outer (bp_col.T @ w_ev)
                    bp_ps = psl.tile([1, 256], f32, space="PSUM", tag="gates")
                    nc.tensor.matmul(out=bp_ps[:], lhsT=bp_c[:, 0, :],
                                     rhs=w_ev[:, 0, :], start=True, stop=False)
                    nc.tensor.matmul(out=bp_ps[:], lhsT=bp_c[:, 1, :],
                                     rhs=w_ev[:, 1, :], start=False, stop=True)
                    bpr = stpool.tile([1, 256], f32, tag="bpr")
                    nc.vector.tensor_copy(out=bpr[:], in_=bp_ps[:])
                    bpf_ps = psl.tile([128, 256], f32, space="PSUM", tag="gates2")
                    nc.tensor.matmul(out=bpf_ps[:], lhsT=ones[:, :],
                                     rhs=bpr[:], start=True, stop=True)
                    nc.vector.tensor_copy(out=bpwf[:], in_=bpf_ps[:])
                elif c == 2:
                    # bgcn replicated to all partitions via outer product
                    bgf_ps = psl.tile([128, 256], f32, space="PSUM", tag="gates2")
                    nc.tensor.matmul(out=bgf_ps[:], lhsT=ones[:, :],
                                     rhs=bgcn[:], start=True, stop=True)
                    bgf = stpool.tile([128, 256], f32, tag="bgf")
                    nc.vector.tensor_copy(out=bgf[:], in_=bgf_ps[:])
                    # t2[:, g, :] = s2d[:, g] * bpwf + bgcn  (per dst block)
                    nc.vector.tensor_tensor(
                        out=t2[:],
                        in0=s2d[:].rearrange("p (g o) -> p g o", o=1)
                            .to_broadcast([128, NGRP, 256]),
                        in1=bpwf[:].rearrange("p (o h) -> p o h", o=1)
                            .to_broadcast([128, NGRP, 256]),
                        op=mybir.AluOpType.mult,
                    )
                    nc.vector.tensor_tensor(
                        out=t2[:],
                        in0=t2[:],
                        in1=bgf[:].rearrange("p (o h) -> p o h", o=1)
                            .to_broadcast([128, NGRP, 256]),
                        op=mybir.AluOpType.add,
                    )

            # ---------- epilogue: out rows = dinv*(xagg@WpW + s2*bpW + dri*bgcn) --
            for t in range(3):
                lo, hi = spans[t]
                nc.scalar.activation(out=xagg[:, lo:hi], in_=pa[t][:],
                                     func=mybir.ActivationFunctionType.Copy)
            for g in range(NGRP):
                ops = psg.tile([128, HID], f32, space="PSUM", tag="ops")
                ds = slice(128 * g, 128 * (g + 1))
                nc.tensor.matmul(out=ops[:], lhsT=xagg[:, ds], rhs=wpw[:],
                                 start=True, stop=True)
                opre = opool.tile([128, HID], f32, tag="opre")
                nc.scalar.activation(out=opre[:], in_=ops[:],
                                     func=mybir.ActivationFunctionType.Copy,
                                     scale=dcol[:, g:g + 1])
                orow = opool.tile([128, HID], f32, tag="orow")
                nc.vector.tensor_tensor(out=orow[:], in0=opre[:],
                                        in1=t2[:, g, :],
                                        op=mybir.AluOpType.add)
                oeng = nc.sync if g % 2 == 0 else nc.scalar
                oeng.dma_start(
                    out=out_t.rearrange("(g p) h -> g p h", p=128)[g],
                    in_=orow[:],
                )

    nc.compile()
    return nc


def _preprocess(edge_index):
    """Host-side graph preprocessing: degrees, serpentine dst sharding, and
    the per-core fp8 edge-multiplicity matrices."""
    import ml_dtypes

    src = np.asarray(edge_index[0], dtype=np.int64)
    dst = np.asarray(edge_index[1], dtype=np.int64)
    loops = np.arange(N_NODES, dtype=np.int64)
    src_all = np.concatenate([src, loops])
    dst_all = np.concatenate([dst, loops])
    deg = np.bincount(dst_all, minlength=N_NODES).astype(np.float64)
    dinv = (1.0 / np.sqrt(deg)).astype(np.float32)

    # serpentine assignment of degree-sorted nodes to cores
    order = np.argsort(-deg, kind="stable")
    r = np.arange(N_NODES)
    rr = r % (2 * M)
    core_r = np.where(rr < M, rr, 2 * M - 1 - rr)
    lrank_r = (r // (2 * M)) * 2 + (rr >= M)
    core_of = np.empty(N_NODES, np.int64)
    lrank_of = np.empty(N_NODES, np.int64)
    core_of[order] = core_r
    lrank_of[order] = lrank_r

    # per-core permutation: perm[c][l] = global node at local rank l
    perm = np.empty((M, N_NODES // M), np.int64)
    perm[core_of[order], lrank_of[order]] = order

    # per-core fp8 multiplicity matrix Mt[p, r*NPC + d] = #edges (128r+p -> d)
    e_core = core_of[dst_all]
    e_dl = lrank_of[dst_all]
    lut = np.arange(256).astype(ml_dtypes.float8_e4m3)
    Mts = []
    for c in range(M):
        sel = e_core == c
        tok = src_all[sel]
        dl = e_dl[sel]
        mt_u8 = np.zeros((128, RANKS * NPC), np.uint8)
        np.add.at(mt_u8, (tok % 128, (tok // 128) * NPC + dl), 1)
        Mts.append(lut[mt_u8])

    # s2[d] = sum over in-edges of dinv[src] (self loop included)
    s2 = np.bincount(dst_all, weights=dinv[src_all].astype(np.float64),
                     minlength=N_NODES).astype(np.float32)

    return dict(dinv=dinv, perm=perm, Mts=Mts, s2=s2)


LAST_RESULT = None


def kernel(x, edge_index, Wp, bp, W_ih, W_hh, b_ih, b_hh, initial_weight, b_gcn):
    global LAST_RESULT
    from concourse.bass_utils import run_bass_kernel_spmd

    x = np.asarray(x, np.float32)
    Wp = np.asarray(Wp, np.float32)
    bp = np.asarray(bp, np.float32)
    W_ih = np.asarray(W_ih, np.float32)
    W_hh = np.asarray(W_hh, np.float32)
    b_ih = np.asarray(b_ih, np.float32)
    b_hh = np.asarray(b_hh, np.float32)
    initial_weight = np.asarray(initial_weight, np.float32)
    b_gcn = np.asarray(b_gcn, np.float32)
    assert x.shape == (N_NODES, IN_DIM)

    pre = _preprocess(edge_index)
    dinv, perm, s2 = pre["dinv"], pre["perm"], pre["s2"]

    if "m" not in _cache:
        _cache["m"] = _build_module()
    nc = _cache["m"]

    # dinv-scaled fp16 source table, token layout (partition n%128, rank n//128)
    xsp = np.zeros((NP, IN_DIM), np.float32)
    xsp[:N_NODES] = x * dinv[:, None]
    xs_tiled = np.ascontiguousarray(
        xsp.reshape(RANKS, 128, IN_DIM).transpose(1, 0, 2)
        .reshape(128, RANKS * 128)).astype(np.float16)
    import ml_dtypes
    shared = {
        "xs_tiled": xs_tiled,
        "WsumT": np.ascontiguousarray((W_ih + W_hh).T).astype(ml_dtypes.bfloat16),
        "IW": initial_weight,
        "IWT": np.ascontiguousarray(initial_weight.T).astype(ml_dtypes.bfloat16),
        "WpT": np.ascontiguousarray(Wp.T),
        "bsum": (b_ih + b_hh).reshape(1, -1),
        "bp_col": np.ascontiguousarray(bp.reshape(-1, 1)),
        "b_gcn": b_gcn.reshape(1, -1),
        "ones_row": np.ones((1, 128), np.float32),
    }
    NLOC = N_NODES // M
    in_maps = []
    for c in range(M):
        pc = perm[c]
        s2dp = np.zeros(NPC, np.float32)
        s2dp[:NLOC] = s2[pc] * dinv[pc]
        dlocp = np.zeros(NPC, np.float32)
        dlocp[:NLOC] = dinv[pc]
        in_maps.append({
            **shared,
            "Mt": pre["Mts"][c],
            "s2d_col": np.ascontiguousarray(s2dp.reshape(NGRP, 128).T),
            "dinv_col": np.ascontiguousarray(dlocp.reshape(NGRP, 128).T),
        })

    res = run_bass_kernel_spmd(nc, in_maps, list(range(M)))
    LAST_RESULT = res

    out = np.empty((N_NODES, HID), np.float32)
    for c in range(M):
        out[perm[c]] = res.results[c]["out"][:NLOC]
    return out
